# revision 16
# baseline (speedup 1.0000x reference)
"""Chamfer distance kernel for Trainium2 — v4.4 (fold-tree + host refine).

v3 was DVE-bound: per 128-row block, two max_index scans over the full
8192-wide bf16 strip (1x mode) cost ~8.7us of the ~11.2us DVE budget.

v4 removes max_index entirely:
- Per (block, direction), the 8192 PSUM f32 columns sit in 8 psum tiles
  of 1024 (bufs=4 -> deep matmul/evac pipelining). 6 tiles are evacuated
  by ACT copies (bf16 strips), 2 by DVE tensor_tensor min against the
  earliest ACT strip (one PSUM input allowed per DVE op), balancing the
  two engines at ~6.4us/block-dir.
- A DVE bf16 fold tree (tensor_tensor at 2x) folds everything to 256
  positions per (block, direction); position s covers the 32 columns
  congruent to s (mod 256).
- Host: per row, argmin over the 256 bf16 position-minima (+ bf16 ties),
  then EXACT f32 recompute of the <=32 candidate columns -> exact idx
  and exact dist for every row. bf16 rounding is monotone, so the column
  achieving the true f32 row-min always lands in a tied-min position.
"""

import numpy as np
import ml_dtypes

import concourse.bacc as bacc
import concourse.mybir as mybir
from concourse import tile
from concourse.bass_utils import run_bass_kernel_spmd

F32 = mybir.dt.float32
BF16 = mybir.dt.bfloat16
AF = mybir.ActivationFunctionType
ALU = mybir.AluOpType
AX = mybir.AxisListType

BF = ml_dtypes.bfloat16

_PROGRAM_CACHE = {}


def _build_program(n_pts=8192, n_cores=8, repeat=1):
    key = (n_pts, n_cores, repeat)
    if key in _PROGRAM_CACHE:
        return _PROGRAM_CACHE[key]

    NB = n_pts // 128          # 64 row blocks
    TW = 2048                  # psum tile width (4 banks)
    NT = n_pts // TW           # 4 psum tiles per (block, dir)
    NPOS = 256                 # final positions per (block, dir)

    nc = bacc.Bacc("TRN2", target_bir_lowering=False, debug=False,
                   num_devices=n_cores)
    uu = nc.dram_tensor("uu", [96, n_pts], BF16, kind="ExternalInput")
    r1o = nc.dram_tensor("r1", [128, NB * NPOS], BF16, kind="ExternalOutput")
    r2o = nc.dram_tensor("r2", [128, NB * NPOS], BF16, kind="ExternalOutput")

    with tile.TileContext(nc) as tc:
        with tc.tile_pool(name="persist", bufs=1) as persist:
            U1 = persist.tile([56, n_pts], BF16, tag="U1")
            U2 = persist.tile([56, n_pts], BF16, tag="U2")
            racc1 = persist.tile([128, NB * NPOS], BF16, tag="racc1")
            racc2 = persist.tile([128, NB * NPOS], BF16, tag="racc2")

            nc.sync.dma_start(U1[0:24, :], uu.ap()[0:24, :])
            nc.sync.dma_start(U1[32:56, :], uu.ap()[24:48, :])
            nc.sync.dma_start(U2[0:24, :], uu.ap()[72:96, :])
            nc.sync.dma_start(U2[32:56, :], uu.ap()[48:72, :])

            dirs = ((U1, U2, 0, racc1),
                    (U2, U1, 32, racc2))

            with tc.tile_pool(name="psum", bufs=4, space="PSUM") as pspool, \
                 tc.tile_pool(name="tt", bufs=3) as tpool, \
                 tc.tile_pool(name="zz", bufs=3) as zpool, \
                 tc.tile_pool(name="vv", bufs=3) as vpool, \
                 tc.tile_pool(name="ww", bufs=3) as wpool, \
                 tc.tile_pool(name="xx", bufs=3) as xpool, \
                 tc.tile_pool(name="yy", bufs=3) as ypool:
                for _ in range(repeat):
                    for nb in range(NB):
                        for di in (0, 1):
                            lhsU, rhsU, base, racc = dirs[di]
                            lhs = lhsU[base:base + 24,
                                       nb * 128:(nb + 1) * 128]
                            T = tpool.tile([128, 3 * TW], BF16, tag="T")
                            C = zpool.tile([128, TW], BF16, tag="C")
                            for t in range(NT):
                                for h in range(2):
                                    ps = pspool.tile([128, TW // 2], F32,
                                                     tag="ps")
                                    for q in range(TW // 1024):
                                        c0 = t * TW + h * 1024 + q * 512
                                        nc.tensor.matmul(
                                            ps[:, q * 512:(q + 1) * 512],
                                            lhs,
                                            rhsU[base:base + 24, c0:c0 + 512],
                                            start=True, stop=True)
                                    if t < 3:
                                        nc.scalar.activation(
                                            T[:, t * TW + h * 1024:
                                              t * TW + (h + 1) * 1024],
                                            ps[:], AF.Copy)
                                    else:
                                        # DVE evac halves: fold tile3 (PSUM)
                                        # with tile0's SBUF strip
                                        nc.vector.tensor_tensor(
                                            C[:, h * 1024:(h + 1) * 1024],
                                            ps[:],
                                            T[:, h * 1024:(h + 1) * 1024],
                                            ALU.min)
                            # bf16 fold tree (DVE 2x) down to NPOS positions
                            V = vpool.tile([128, TW], BF16, tag="V")
                            nc.vector.tensor_tensor(
                                V[:], T[:, TW:2 * TW], T[:, 2 * TW:3 * TW],
                                ALU.min)
                            W = wpool.tile([128, TW], BF16, tag="W")
                            nc.vector.tensor_tensor(
                                W[:], C[:], V[:], ALU.min)
                            X = xpool.tile([128, 1024], BF16, tag="X")
                            nc.vector.tensor_tensor(
                                X[:], W[:, 0:1024], W[:, 1024:2048], ALU.min)
                            Y = ypool.tile([128, 512], BF16, tag="Y")
                            nc.vector.tensor_tensor(
                                Y[:], X[:, 0:512], X[:, 512:1024], ALU.min)
                            nc.vector.tensor_tensor(
                                racc[:, nb * NPOS:(nb + 1) * NPOS],
                                Y[:, 0:256], Y[:, 256:512], ALU.min)

            nc.sync.dma_start(r1o.ap(), racc1[:])
            nc.sync.dma_start(r2o.ap(), racc2[:])

    nc.compile()
    _PROGRAM_CACHE[key] = nc
    return nc


def _split3(v):
    h = v.astype(BF).astype(np.float32)
    r = (v - h).astype(np.float32)
    m = r.astype(BF).astype(np.float32)
    l = (r - m).astype(BF).astype(np.float32)
    return h, m, l


def _forms(xyz):
    """[N,3] f32 -> (A, B) [24, N] bf16 triple-split homogeneous forms."""
    x = np.ascontiguousarray(xyz.T).astype(np.float32)
    n = (x * x).sum(0, dtype=np.float32)[None, :]
    s = (-2.0 * x).astype(np.float32)
    sh, sm, sl = _split3(s)
    xh, xm, xl = _split3(x)
    nh, nm, nl = _split3(n)
    ones = np.ones_like(n)
    A = np.concatenate([sh, sh, sm, sh, sl, sm, ones, ones, ones,
                        nh, nm, nl]).astype(BF)
    Bf = np.concatenate([xh, xm, xh, xl, xh, xm, nh, nm, nl,
                         ones, ones, ones]).astype(BF)
    return A, Bf


def _fold_maps(n_pts=8192):
    """colmap[col] = final position (0..63) within a (block, dir);
    cands[s] = ascending array of the 128 columns folded into position s.

    Fold structure: W[m] = min over cols congruent to m (mod 2048), then
    halvings down to 256 -> position s = col % 256."""
    cols = np.arange(n_pts)
    s = cols % 256
    cands = np.empty((256, 32), np.int64)
    for p in range(256):
        cc = np.nonzero(s == p)[0]
        assert cc.size == 32
        cands[p] = cc
    return s, cands


_COLMAP, _CANDS = _fold_maps()


def _refine(R, Xq, Xc):
    """R: [128, NB*64] bf16 position minima for one (batch, dir).
    Xq: query points [N, 3], Xc: candidate cloud [M, 3].
    Returns exact (dist [N], idx [N]) via f32 recompute of candidates."""
    npos = _CANDS.shape[0]
    NB = R.shape[1] // npos
    N = NB * 128
    # row n = b*128 + p  ->  vals[n] = R[p, b*npos:(b+1)*npos]
    vals = np.asarray(R).reshape(128, NB, npos) \
        .transpose(1, 0, 2).reshape(N, npos)
    vf = vals.astype(np.float32)
    m = vf.min(1)
    smin = vf.argmin(1)
    nties = (vf == m[:, None]).sum(1)
    cands = _CANDS[smin]                                   # [N, ncand]
    nc2 = (Xc * Xc).sum(1)                                 # [M]
    nq = (Xq * Xq).sum(1)                                  # [N]
    cpts = Xc[cands]                                       # [N, 128, 3]
    d = nq[:, None] + nc2[cands] \
        - 2.0 * np.einsum('nd,nkd->nk', Xq, cpts)
    d = np.maximum(d.astype(np.float32), 0.0)
    loc = d.argmin(1)
    idx = cands[np.arange(N), loc]
    dist = d[np.arange(N), loc]
    # rows where several positions tie at the bf16 min: search their union
    rows = np.nonzero(nties > 1)[0]
    for r in rows:
        ss = np.nonzero(vf[r] == m[r])[0]
        cc = np.sort(np.concatenate([_CANDS[s] for s in ss]))
        dd = nq[r] + nc2[cc] - 2.0 * (Xc[cc] @ Xq[r])
        dd = np.maximum(dd.astype(np.float32), 0.0)
        l = dd.argmin()
        idx[r] = cc[l]
        dist[r] = dd[l]
    return dist, idx.astype(np.int32)


def kernel(xyz1: np.ndarray, xyz2: np.ndarray, repeat: int = 1):
    xyz1 = np.asarray(xyz1, dtype=np.float32)
    xyz2 = np.asarray(xyz2, dtype=np.float32)
    B, N, _ = xyz1.shape
    M = xyz2.shape[1]
    assert B == 8 and N == 8192 and M == 8192, (B, N, M)

    nc = _build_program(N, B, repeat)

    in_maps = []
    for b in range(B):
        A1, B1 = _forms(xyz1[b])
        A2, B2 = _forms(xyz2[b])
        in_maps.append({"uu": np.concatenate([A1, B1, A2, B2])})
    res = run_bass_kernel_spmd(nc, in_maps, list(range(B)))

    dist1 = np.empty((B, N), np.float32)
    dist2 = np.empty((B, M), np.float32)
    idx1 = np.empty((B, N), np.int32)
    idx2 = np.empty((B, M), np.int32)
    for b in range(B):
        r = res.results[b]
        dist1[b], idx1[b] = _refine(np.asarray(r["r1"]), xyz1[b], xyz2[b])
        dist2[b], idx2[b] = _refine(np.asarray(r["r2"]), xyz2[b], xyz1[b])
    return dist1, dist2, idx1, idx2
